# revision 1
# baseline (speedup 1.0000x reference)
"""Trainium2 8-core kernel for the Contrast module:

    za_p = ELU(za @ W1 + b1) @ W2 + b2          (same for zb)
    za_ca = softmax((za_p Wq + bq)(zb_p Wk + bk)^T / sqrt(256)) @ (zb_p Wv + bv)
    zb_ca = softmax((zb_p Wq + bq)(za_p Wk + bk)^T / sqrt(256)) @ (za_p Wv + bv)
    out = concat(za_ca, zb_ca, axis=1)

Sharding: rows (N axis) data-parallel across 8 cores; weights replicated.
Each core projects its 1024-row slice of za and zb, computes its K/V
shards, AllGathers K and V separately (4 collectives, overlapped with
compute; scores start as soon as the K gather lands), and runs
its [1024, 8192] attention block for both directions.

Layout notes:
 - All activations flow feature-major ("transposed"): inputs arrive as
   zaT [h, n] so every matmul contracts over the partition axis without
   any on-chip transposes.  out = lhsT.T @ rhs with
     hT = W1^T zaT, pT = W2^T hT, QT/KT = W^T pT (feature-major)
     V  = pT as lhsT with Wv as rhs (token-major)
     scoresT[k, q] = (KT slice)^T @ QT, attn@V = expT slice as lhsT, V as rhs.
 - f32r (FP32 data, FP22 multiply) matmuls: full PE rate at moving dim>=256.
 - softmax denominator: V panels carry a ones column; attn@V is split into
   N=256 and N=258 matmuls so the rowsum accumulates in PSUM col 512.
 - No max-subtraction in softmax: scores are ~N(0, 0.85^2), exp is safe.
 - ELU+1 = max(x+1, min(exp(x), 1)); the -1 is folded into b2 on the host.
 - 1/16 score scale folded into Wq/bq on the host.
"""

import numpy as np

import concourse.mybir as mybir
import concourse.tile as tile
from concourse import bacc
from concourse.bass_utils import run_bass_kernel_spmd

dt = mybir.dt
AF = mybir.ActivationFunctionType
ALU = mybir.AluOpType

R = 8            # cores
N = 8192         # total rows
H = 1024         # hidden
D = 512          # attention dim
NL = N // R      # rows per core
HC = H // 128    # 8 h-chunks
DC = D // 128    # 4 d-chunks
NB = NL // 512   # 2 n-blocks per core slice
SCALE = 16.0     # sqrt(512/2)
KVF = D * NL     # floats per K (or V) shard

F32R = dt.float32r


def _r(ap):
    return ap.bitcast(F32R)


def build():
    nc = bacc.Bacc("TRN2", target_bir_lowering=False, debug=False, num_devices=R)

    def inp(name, shape):
        return nc.dram_tensor(name, shape, dt.float32, kind="ExternalInput")

    zT = {"a": inp("zaT", [128, HC, NL]), "b": inp("zbT", [128, HC, NL])}
    w1 = inp("W1t", [128, HC, H])
    w2 = inp("W2t", [128, HC, D])
    wq = inp("Wqt", [128, DC, D])
    wk = inp("Wkt", [128, DC, D])
    wv = inp("Wvt", [128, DC, D])
    b1d = inp("b1t", [128, HC])
    b1p1d = inp("b1p1t", [128, HC])
    b2d = inp("b2t", [128, DC])
    bqd = inp("bqt", [128, DC])
    bkd = inp("bkt", [128, DC])
    bvd = inp("bvt", [128, D])
    vpadd = inp("vpad", [128, 2 * HC])
    out_d = nc.dram_tensor("out", [NL, 2 * D], dt.float32, kind="ExternalOutput")

    with tile.TileContext(nc) as tc:
        psum = tc.alloc_tile_pool(name="psum", bufs=1, space="PSUM")
        dram = tc.alloc_tile_pool(name="dram", bufs=1, space="DRAM")
        const = tc.alloc_tile_pool(name="const", bufs=1)
        qtp = tc.alloc_tile_pool(name="qtp", bufs=1)
        wkvp = tc.alloc_tile_pool(name="wkvp", bufs=1)
        projp = tc.alloc_tile_pool(name="projp", bufs=1)

        # ---- constants ----
        b1 = const.tile([128, HC], dt.float32, name="b1")
        b1p1 = const.tile([128, HC], dt.float32, name="b1p1")
        b2 = const.tile([128, DC], dt.float32, name="b2")
        bq = const.tile([128, DC], dt.float32, name="bq")
        bk = const.tile([128, DC], dt.float32, name="bk")
        bv = const.tile([128, D], dt.float32, name="bv")
        for t, d_ in ((b1, b1d), (b1p1, b1p1d), (b2, b2d), (bq, bqd), (bk, bkd), (bv, bvd)):
            nc.sync.dma_start(t[:], d_.ap())
        wqt = wkvp.tile([128, DC, D], F32R, name="wqt")
        wkt = wkvp.tile([128, DC, D], F32R, name="wkt")
        wvt = wkvp.tile([128, DC, D], F32R, name="wvt")
        for t, d_ in ((wqt, wq), (wkt, wk), (wvt, wv)):
            nc.sync.dma_start(t[:], _r(d_.ap()))

        # ---- projection weights (chunked DMA so PE can start early) ----
        w1t = projp.tile([128, HC, H], F32R, name="w1t")
        for hc in range(HC):
            nc.sync.dma_start(w1t[:, hc, :], _r(w1.ap()[:, hc, :]))
        w2t = projp.tile([128, HC, D], F32R, name="w2t")
        nc.sync.dma_start(w2t[:], _r(w2.ap()))

        pT = {
            "a": wkvp.tile([128, DC, NL], F32R, name="pta"),
            "b": wkvp.tile([128, DC, NL], F32R, name="ptb"),
        }

        # AG buffers: direction X's attention consumes K/V derived from the
        # OTHER projection; ag_for[X] is filled from pT[other(X)].
        agin_k = {}
        agin_v = {}
        agout_k = {}
        agout_v = {}
        for x in ("b", "a"):
            agin_k[x] = dram.tile([KVF], dt.float32, name=f"agink_{x}")
            agin_v[x] = dram.tile([KVF], dt.float32, name=f"aginv_{x}")
            agout_k[x] = dram.tile(
                [R * KVF], dt.float32, name=f"agoutk_{x}", addr_space="Shared"
            )
            agout_v[x] = dram.tile(
                [R * KVF], dt.float32, name=f"agoutv_{x}", addr_space="Shared"
            )

        # ================= projection + K/V shards =================
        for src, other in (("a", "b"), ("b", "a")):
            for nb in range(NB):
                ns = slice(nb * 512, (nb + 1) * 512)
                z = projp.tile([128, HC, 512], F32R, tag="z", bufs=2, name=f"z_{src}{nb}")
                nc.sync.dma_start(z[:], _r(zT[src].ap()[:, :, ns]))
                hT = projp.tile([128, HC, 512], F32R, tag="h", bufs=1, name=f"h_{src}{nb}")
                for d1c in range(HC):
                    ps = psum.tile([128, 512], dt.float32, tag="mm", bufs=4, name="ps_h")
                    for hc in range(HC):
                        nc.tensor.matmul(
                            ps[:],
                            w1t[:, hc, d1c * 128 : (d1c + 1) * 128],
                            z[:, hc, :],
                            start=(hc == 0),
                            stop=(hc == HC - 1),
                        )
                    # ELU(x)+1 = max(x+1, min(exp(x), 1)), x = ps + b1
                    e = projp.tile([128, 512], dt.float32, tag="e", bufs=2, name="e")
                    nc.scalar.activation(e[:], ps[:], AF.Exp, bias=b1[:, d1c : d1c + 1])
                    xp1 = projp.tile([128, 512], dt.float32, tag="xp1", bufs=2, name="xp1")
                    nc.vector.tensor_scalar(
                        xp1[:], ps[:], b1p1[:, d1c : d1c + 1], None, ALU.add
                    )
                    nc.vector.tensor_scalar(e[:], e[:], 1.0, None, ALU.min)
                    nc.vector.tensor_tensor(hT[:, d1c, :], xp1[:], e[:], ALU.max)
                for d2c in range(DC):
                    ps = psum.tile([128, 512], dt.float32, tag="mm", bufs=4, name="ps_p")
                    for d1c in range(HC):
                        nc.tensor.matmul(
                            ps[:],
                            w2t[:, d1c, d2c * 128 : (d2c + 1) * 128],
                            hT[:, d1c, :],
                            start=(d1c == 0),
                            stop=(d1c == HC - 1),
                        )
                    nc.scalar.activation(
                        pT[src][:, d2c, ns], ps[:], AF.Identity, bias=b2[:, d2c : d2c + 1]
                    )

            # K/V shards for the *other* direction, written to AG input
            ktv = agin_k[other][:].rearrange("(d n) -> d n", n=NL)
            vv = agin_v[other][:].rearrange("(n d) -> n d", d=D)
            for dc in range(DC):
                for nb in range(NB):
                    ps = psum.tile([128, 512], dt.float32, tag="mm", bufs=4, name="ps_k")
                    for d2c in range(DC):
                        nc.tensor.matmul(
                            ps[:],
                            wkt[:, d2c, dc * 128 : (dc + 1) * 128],
                            pT[src][:, d2c, nb * 512 : (nb + 1) * 512],
                            start=(d2c == 0),
                            stop=(d2c == DC - 1),
                        )
                    s = projp.tile([128, 512], dt.float32, tag="stg", bufs=3, name="stg_k")
                    nc.scalar.activation(s[:], ps[:], AF.Identity, bias=bk[:, dc : dc + 1])
                    nc.sync.dma_start(
                        ktv[dc * 128 : (dc + 1) * 128, nb * 512 : (nb + 1) * 512], s[:]
                    )
            nc.gpsimd.collective_compute(
                "AllGather",
                ALU.bypass,
                ins=[agin_k[other].opt()],
                outs=[agout_k[other].opt()],
                replica_groups=[list(range(R))],
            )
            for nt in range(NL // 128):
                ps = psum.tile([128, 512], dt.float32, tag="mm", bufs=4, name="ps_v")
                for d2c in range(DC):
                    nc.tensor.matmul(
                        ps[:],
                        pT[src][:, d2c, nt * 128 : (nt + 1) * 128],
                        wvt[:, d2c, :],
                        start=(d2c == 0),
                        stop=(d2c == DC - 1),
                    )
                s = projp.tile([128, 512], dt.float32, tag="stg", bufs=3, name="stg_v")
                nc.scalar.activation(s[:], ps[:], AF.Copy)
                nc.sync.dma_start(vv[nt * 128 : (nt + 1) * 128, :], s[:])
            nc.gpsimd.collective_compute(
                "AllGather",
                ALU.bypass,
                ins=[agin_v[other].opt()],
                outs=[agout_v[other].opt()],
                replica_groups=[list(range(R))],
            )

        projp.release()

        # ================= queries =================
        qT = {}
        for x in ("b", "a"):
            qT[x] = qtp.tile([128, DC, NL], F32R, name=f"qt_{x}")
            for dc in range(DC):
                for nb in range(NB):
                    ps = psum.tile([128, 512], dt.float32, tag="mm", bufs=4, name="ps_q")
                    for d2c in range(DC):
                        nc.tensor.matmul(
                            ps[:],
                            wqt[:, d2c, dc * 128 : (dc + 1) * 128],
                            pT[x][:, d2c, nb * 512 : (nb + 1) * 512],
                            start=(d2c == 0),
                            stop=(d2c == DC - 1),
                        )
                    nc.scalar.activation(
                        qT[x][:, dc, nb * 512 : (nb + 1) * 512],
                        ps[:],
                        AF.Identity,
                        bias=bq[:, dc : dc + 1],
                    )
        wkvp.release()

        # ================= attention =================
        attnp = tc.alloc_tile_pool(name="attnp", bufs=1)
        for x, col in (("b", 1), ("a", 0)):
            accs = {}
            for r in range(R):
                base = r * KVF
                ktile = attnp.tile([128, DC, NL], F32R, tag="kt", bufs=2, name=f"kt{r}")
                nc.sync.dma_start(
                    ktile[:],
                    _r(
                        agout_k[x][base : base + KVF].rearrange(
                            "(dc p n) -> p dc n", p=128, n=NL
                        )
                    ),
                )
                vtile = attnp.tile(
                    [128, NL // 128, D + 2], F32R, tag="vt", bufs=2, name=f"vt{r}"
                )
                nc.sync.dma_start(
                    vtile[:, :, 0:D],
                    _r(
                        agout_v[x][base : base + KVF].rearrange(
                            "(kc p d) -> p kc d", p=128, d=D
                        )
                    ),
                )
                nc.sync.dma_start(
                    vtile[:, :, D : D + 2],
                    _r(vpadd.ap().rearrange("p (kc c) -> p kc c", c=2)),
                )
                for qb in range(NB):
                    qs = slice(qb * 512, (qb + 1) * 512)
                    exps = []
                    for kt_i in range(NL // 128):
                        ps = psum.tile(
                            [128, 512], dt.float32, tag="mm", bufs=4, name="ps_s"
                        )
                        for dc in range(DC):
                            nc.tensor.matmul(
                                ps[:],
                                ktile[:, dc, kt_i * 128 : (kt_i + 1) * 128],
                                qT[x][:, dc, qs],
                                start=(dc == 0),
                                stop=(dc == DC - 1),
                            )
                        ex = attnp.tile(
                            [128, 512], F32R, tag="exp", bufs=32, name=f"ex{kt_i}"
                        )
                        nc.scalar.activation(ex[:], ps[:], AF.Exp)
                        exps.append(ex)
                    for qt_i in range(4):
                        qsl = slice(qt_i * 128, (qt_i + 1) * 128)
                        p1 = psum.tile(
                            [128, 256], dt.float32, tag="po1", bufs=2, name="po1"
                        )
                        p2 = psum.tile(
                            [128, 258], dt.float32, tag="po2", bufs=2, name="po2"
                        )
                        for kc in range(NL // 128):
                            nc.tensor.matmul(
                                p1[:],
                                exps[kc][:, qsl],
                                vtile[:, kc, 0:256],
                                start=(kc == 0),
                                stop=(kc == NL // 128 - 1),
                            )
                            nc.tensor.matmul(
                                p2[:],
                                exps[kc][:, qsl],
                                vtile[:, kc, 256 : D + 2],
                                start=(kc == 0),
                                stop=(kc == NL // 128 - 1),
                            )
                        if r == 0:
                            acc = attnp.tile(
                                [128, D + 2], dt.float32, tag="acc", bufs=8,
                                name=f"acc{qb}{qt_i}",
                            )
                            accs[(qb, qt_i)] = acc
                            nc.vector.tensor_copy(acc[:, 0:256], p1[:])
                            nc.vector.tensor_copy(acc[:, 256 : D + 2], p2[:])
                        else:
                            acc = accs[(qb, qt_i)]
                            nc.vector.tensor_tensor(
                                acc[:, 0:256], acc[:, 0:256], p1[:], ALU.add
                            )
                            nc.vector.tensor_tensor(
                                acc[:, 256 : D + 2], acc[:, 256 : D + 2], p2[:], ALU.add
                            )
            # finalize: out = acc[:, :512] / acc[:, 512] + bv
            for qb in range(NB):
                for qt_i in range(4):
                    acc = accs[(qb, qt_i)]
                    rr = attnp.tile([128, 1], dt.float32, tag="rr", bufs=4, name="rr")
                    nc.vector.reciprocal(rr[:], acc[:, D : D + 1])
                    ot = attnp.tile([128, D], dt.float32, tag="ot", bufs=3, name="ot")
                    nc.vector.tensor_scalar(ot[:], acc[:, 0:D], rr[:], None, ALU.mult)
                    nc.vector.tensor_tensor(ot[:], ot[:], bv[:], ALU.add)
                    r0 = qb * 512 + qt_i * 128
                    nc.sync.dma_start(
                        out_d.ap()[r0 : r0 + 128, col * D : (col + 1) * D], ot[:]
                    )
        attnp.release()
        qtp.release()
        const.release()
        dram.release()
        psum.release()

    nc.compile()
    return nc


_NC = None


def _get_nc():
    global _NC
    if _NC is None:
        _NC = build()
    return _NC


def _chunk_w(w):
    """[X, Y] -> [128, X//128, Y] partition-chunked, contiguous."""
    x, y = w.shape
    return np.ascontiguousarray(w.reshape(x // 128, 128, y).transpose(1, 0, 2))


def _chunk_b(b):
    return np.ascontiguousarray(np.asarray(b, np.float32).reshape(-1, 128).T)


def prep_in_maps(za, zb, W1, b1, W2, b2, Wq, bq, Wk, bk, Wv, bv):
    za = np.asarray(za, np.float32)
    zb = np.asarray(zb, np.float32)
    W1 = np.asarray(W1, np.float32)
    W2 = np.asarray(W2, np.float32)
    Wq = np.asarray(Wq, np.float32)
    Wk = np.asarray(Wk, np.float32)
    Wv = np.asarray(Wv, np.float32)
    b1 = np.asarray(b1, np.float32)
    b2 = np.asarray(b2, np.float32)
    bq = np.asarray(bq, np.float32)
    bk = np.asarray(bk, np.float32)
    bv = np.asarray(bv, np.float32)

    shared = {
        "W1t": _chunk_w(W1),
        "W2t": _chunk_w(W2),
        "Wqt": _chunk_w(Wq / SCALE),
        "Wkt": _chunk_w(Wk),
        "Wvt": _chunk_w(Wv),
        "b1t": _chunk_b(b1),
        "b1p1t": _chunk_b(b1 + 1.0),
        "b2t": _chunk_b(b2 - W2.sum(axis=0)),
        "bqt": _chunk_b(bq / SCALE),
        "bkt": _chunk_b(bk),
        "bvt": np.ascontiguousarray(np.broadcast_to(bv, (128, D)).astype(np.float32)),
        "vpad": np.ascontiguousarray(
            np.broadcast_to(np.tile(np.array([1.0, 0.0], np.float32), HC), (128, 2 * HC))
        ),
    }
    zaT = np.ascontiguousarray(za.T)  # [H, N]
    zbT = np.ascontiguousarray(zb.T)
    in_maps = []
    for c in range(R):
        cs = slice(c * NL, (c + 1) * NL)
        in_maps.append(
            {
                "zaT": _chunk_w(zaT[:, cs]),
                "zbT": _chunk_w(zbT[:, cs]),
                **shared,
            }
        )
    return in_maps


def kernel(**inputs) -> np.ndarray:
    nc = _get_nc()
    in_maps = prep_in_maps(**inputs)
    res = run_bass_kernel_spmd(nc, in_maps, core_ids=list(range(R)))
    return np.concatenate([res.results[c]["out"] for c in range(R)], axis=0)



# revision 4
# speedup vs baseline: 1.1904x; 1.1904x over previous
"""Trainium2 8-core collective-free kernel for the Contrast module.

    p_x  = ELU(x @ W1 + b1) @ W2 + b2                     (x in {za, zb})
    S_a  = softmax-scores for dir a = (Q_a K_b^T)/s ;  out_a = softmax(S_a) @ V_b
    out  = concat(out_a, out_b, axis=1)

Key algebra (per direction, q rows own to this core, keys = other input):
    Q K^T/s = p_q (Wq Wk^T/s) p_k^T + (bq Wk^T/s) p_k^T + rowconst(q)
    rowconst cancels in softmax  =>  S = A p_k^T  with  A = p_q M + u,
        M = Wq Wk^T / s,  u = bq Wk^T / s      (folded on host)
    attn @ V = (E @ p_k) Wv / denom + bv       (E = exp(S), denom = E @ 1)

So K and V are never materialized. Every core is fully independent:
it projects ALL rows of za and zb itself (8x redundant compute, but
zero collectives / cross-core sync -> immune to launch skew and
collective latency) and computes attention for its own 1024-row slice.

Layout: feature-major ("transposed") activations as in-flows,
token-major p (for E@p) obtained via PE transposes. f32r everywhere.
"""

import numpy as np

import concourse.mybir as mybir
import concourse.tile as tile
from concourse import bacc
from concourse.bass_utils import run_bass_kernel_spmd

dt = mybir.dt
AF = mybir.ActivationFunctionType
ALU = mybir.AluOpType

R = 8            # cores
N = 8192         # total rows
H = 1024         # hidden
D = 512          # attention dim
NL = N // R      # own rows per core
HC = H // 128    # 8 h-chunks
DC = D // 128    # 4 d-chunks
KB = 512         # key-block rows
NKB = N // KB    # 16 key blocks
SCALE = 16.0     # sqrt(512/2)

F32R = dt.float32r
W1_BF16 = True      # W1/z in bf16: same PE rate, half the DMA, frees SBUF
ZDT = dt.bfloat16 if W1_BF16 else dt.float32


def _r(ap):
    return ap.bitcast(F32R)


def build():
    nc = bacc.Bacc("TRN2", target_bir_lowering=False, debug=False, num_devices=R)

    def inp(name, shape, dty=dt.float32):
        return nc.dram_tensor(name, shape, dty, kind="ExternalInput")

    # full inputs, replicated on every core
    zT = {"a": inp("zaT", [128, HC, N], ZDT), "b": inp("zbT", [128, HC, N], ZDT)}
    # own-slice inputs (per-core)
    zq = {"a": inp("zqa", [128, HC, NL], ZDT), "b": inp("zqb", [128, HC, NL], ZDT)}
    w1 = inp("W1t", [128, HC, H], ZDT)
    w2 = inp("W2t", [128, HC, D])
    mt_d = inp("Mt", [128, DC, D], dt.bfloat16)
    wv_d = inp("Wvt", [128, DC, D])
    b1d = inp("b1t", [128, HC])
    b1p1d = inp("b1p1t", [128, HC])
    b2d = inp("b2t", [128, DC])
    utd = inp("ut", [128, DC])
    bvd = inp("bvt", [128, D])
    identd = inp("ident", [128, 128])
    identbd = inp("identb", [128, 128], dt.bfloat16)
    vpadd = inp("vpad", [128, 2 * DC], dt.bfloat16)
    out_d = nc.dram_tensor("out", [NL, 2 * D], dt.float32, kind="ExternalOutput")

    with tile.TileContext(nc) as tc:
        psum = tc.alloc_tile_pool(name="psum", bufs=1, space="PSUM")
        const = tc.alloc_tile_pool(name="const", bufs=1)
        persist = tc.alloc_tile_pool(name="persist", bufs=1)
        stream = tc.alloc_tile_pool(name="stream", bufs=1)

        # ---- small constants (tiles only; DMAs emitted after the hot
        # w1t/z interleave below) ----
        b1 = const.tile([128, HC], dt.float32, name="b1")
        b1p1 = const.tile([128, HC], dt.float32, name="b1p1")
        b2 = const.tile([128, DC], dt.float32, name="b2")
        ut = const.tile([128, DC], dt.float32, name="ut")
        bv = const.tile([128, D], dt.float32, name="bv")
        ident = const.tile([128, 128], F32R, name="ident")
        identb = const.tile([128, 128], dt.bfloat16, name="identb")

        w1t = persist.tile([128, HC, H], ZDT, name="w1t")
        w2t = persist.tile([128, HC, D], F32R, name="w2t")
        mt = persist.tile([128, DC, D], dt.bfloat16, name="mt")
        wvt = persist.tile([128, DC, D], F32R, name="wvt")

        # A^T per direction: [d, q] feature-major, own 1024 q rows
        AT = {
            x: persist.tile([128, DC, NL], dt.bfloat16, name=f"at_{x}") for x in ("a", "b")
        }
        # numerator accumulators (token-major [q, d] + denom col 512)
        acc = {
            x: persist.tile([128, NL // 128, D + 2], dt.float32, name=f"acc_{x}")
            for x in ("a", "b")
        }

        # ---------- helpers ----------
        def prefetch_z(z_tensor, n0, label):
            zch = []
            for hc in range(HC):
                zt = stream.tile([128, KB], ZDT, tag="z", bufs=8, name=f"z{label}")
                nc.sync.dma_start(zt[:], z_tensor.ap()[:, hc, n0 : n0 + KB])
                zch.append(zt)
            return zch

        def elu(ps, d1c, hT):
            # ELU(x)+1 = max(x+1, min(exp(x), 1)), x = ps + b1
            # both PSUM reads on scalar: cross-engine PSUM reads serialize
            e = stream.tile([128, KB], dt.float32, tag="exp1", bufs=2, name="e")
            nc.scalar.activation(e[:], ps[:], AF.Exp, bias=b1[:, d1c : d1c + 1])
            xp1 = stream.tile([128, KB], dt.float32, tag="exp1", bufs=2, name="xp1")
            nc.scalar.activation(
                xp1[:], ps[:], AF.Identity, bias=b1p1[:, d1c : d1c + 1]
            )
            nc.vector.tensor_scalar(e[:], e[:], 1.0, None, ALU.min)
            nc.vector.tensor_tensor(hT[:, d1c, :], xp1[:], e[:], ALU.max)

        def proj_block(z_tensor, n0, label, zch=None, hc_outer=False):
            """W1+ELU+W2: project rows [n0, n0+KB) -> pT [128, DC, 512] f32r."""
            if zch is None:
                zch = prefetch_z(z_tensor, n0, label)
            hT = stream.tile([128, HC, KB], F32R, tag="h", bufs=1, name="hT")
            if hc_outer:
                # W1 consumes z/w1t chunk-by-chunk: PE starts as soon as the
                # first chunks land instead of waiting for all of them
                for half in range(2):
                    pss = [
                        psum.tile([128, KB], dt.float32, tag="mm", bufs=4,
                                  name="ps_h")
                        for _ in range(4)
                    ]
                    for hc in range(HC):
                        for j in range(4):
                            d1c = half * 4 + j
                            nc.tensor.matmul(
                                pss[j][:],
                                w1t[:, hc, d1c * 128 : (d1c + 1) * 128],
                                zch[hc][:],
                                start=(hc == 0),
                                stop=(hc == HC - 1),
                            )
                    for j in range(4):
                        elu(pss[j], half * 4 + j, hT)
            else:
                for d1c in range(HC):
                    ps = psum.tile([128, KB], dt.float32, tag="mm", bufs=4,
                                   name="ps_h")
                    for hc in range(HC):
                        nc.tensor.matmul(
                            ps[:],
                            w1t[:, hc, d1c * 128 : (d1c + 1) * 128],
                            zch[hc][:],
                            start=(hc == 0),
                            stop=(hc == HC - 1),
                        )
                    elu(ps, d1c, hT)
            pT = stream.tile([128, DC, KB], dt.bfloat16, tag="pt", bufs=4, name="pT")
            for d2c in range(DC):
                ps = psum.tile([128, KB], dt.float32, tag="mm", bufs=4, name="ps_p")
                for hc in range(HC):
                    nc.tensor.matmul(
                        ps[:],
                        w2t[:, hc, d2c * 128 : (d2c + 1) * 128],
                        hT[:, hc, :],
                        start=(hc == 0),
                        stop=(hc == HC - 1),
                    )
                nc.scalar.activation(
                    pT[:, d2c, :], ps[:], AF.Identity, bias=b2[:, d2c : d2c + 1]
                )
            return pT

        def attn_block(pT, dirx, first):
            """One key block vs all my q rows for direction dirx."""
            # token-major P via PE transposes, pad cols 512..514 = [1, 0]
            P = stream.tile([128, DC, D + 2], dt.bfloat16, tag="P", bufs=2, name="P")
            nc.sync.dma_start(
                P[:, :, D : D + 2],
                vpadd.ap().rearrange("p (kc c) -> p kc c", c=2),
            )
            for kc in range(DC):
                tp = psum.tile([128, D], dt.bfloat16, tag="tp", bufs=2, name="tp")
                for dc in range(DC):
                    nc.tensor.transpose(
                        tp[:, dc * 128 : (dc + 1) * 128],
                        pT[:, dc, kc * 128 : (kc + 1) * 128],
                        identb[:],
                    )
                nc.vector.tensor_copy(P[:, kc, 0:D], tp[:])
            for qb in range(NL // KB):
                qs = slice(qb * KB, (qb + 1) * KB)
                exps = []
                for kt in range(DC):
                    ps = psum.tile([128, KB], dt.float32, tag="mm", bufs=4, name="ps_s")
                    for dc in range(DC):
                        nc.tensor.matmul(
                            ps[:],
                            pT[:, dc, kt * 128 : (kt + 1) * 128],
                            AT[dirx][:, dc, qs],
                            start=(dc == 0),
                            stop=(dc == DC - 1),
                        )
                    ex = stream.tile([128, KB], dt.bfloat16, tag="ex", bufs=6, name="ex")
                    nc.scalar.activation(ex[:], ps[:], AF.Exp)
                    exps.append(ex)
                for qt in range(KB // 128):
                    qsl = slice(qt * 128, (qt + 1) * 128)
                    qg = qb * (KB // 128) + qt
                    p1 = psum.tile([128, 256], dt.float32, tag="po1", bufs=1, name="po1")
                    p2 = psum.tile([128, D + 2 - 256], dt.float32, tag="po2", bufs=1,
                                   name="po2")
                    for kc in range(DC):
                        nc.tensor.matmul(
                            p1[:], exps[kc][:, qsl], P[:, kc, 0:256],
                            start=(kc == 0), stop=(kc == DC - 1),
                        )
                        nc.tensor.matmul(
                            p2[:], exps[kc][:, qsl], P[:, kc, 256 : D + 2],
                            start=(kc == 0), stop=(kc == DC - 1),
                        )
                    a = acc[dirx]
                    if first:
                        nc.vector.tensor_copy(a[:, qg, 0:256], p1[:])
                        nc.vector.tensor_copy(a[:, qg, 256 : D + 2], p2[:])
                    else:
                        nc.vector.tensor_tensor(a[:, qg, 0:256], a[:, qg, 0:256],
                                                p1[:], ALU.add)
                        nc.vector.tensor_tensor(a[:, qg, 256 : D + 2],
                                                a[:, qg, 256 : D + 2], p2[:], ALU.add)

        # ---------- phase 1: own projections -> A ----------
        # interleaved w1t/z chunk pairs so the hc-outer first W1 block can
        # consume chunk hc as soon as its pair lands
        z_first = []
        for hc in range(HC):
            nc.sync.dma_start(w1t[:, hc, :], w1.ap()[:, hc, :])
            zt = stream.tile([128, KB], ZDT, tag="z", bufs=8, name="zqa0")
            nc.sync.dma_start(zt[:], zq["a"].ap()[:, hc, 0:KB])
            z_first.append(zt)
        for t, d_ in ((b1, b1d), (b1p1, b1p1d), (b2, b2d), (ut, utd), (bv, bvd)):
            nc.sync.dma_start(t[:], d_.ap())
        nc.sync.dma_start(ident[:], _r(identd.ap()))
        nc.sync.dma_start(identb[:], identbd.ap())
        for hc in range(HC):
            nc.sync.dma_start(w2t[:, hc, :], _r(w2.ap()[:, hc, :]))
        nc.sync.dma_start(mt[:], mt_d.ap())

        own_pts = {}
        for src, nb in (("a", 0), ("b", 0), ("b", 1), ("a", 1)):
            pT = proj_block(
                zq[src], nb * KB, f"q{src}{nb}",
                zch=(z_first if (src, nb) == ("a", 0) else None),
                hc_outer=((src, nb) == ("a", 0)),
            )
            own_pts[(src, nb)] = pT
            for dc in range(DC):
                ps = psum.tile([128, KB], dt.float32, tag="mm", bufs=4, name="ps_a")
                for d2c in range(DC):
                    nc.tensor.matmul(
                        ps[:],
                        mt[:, d2c, dc * 128 : (dc + 1) * 128],
                        pT[:, d2c, :],
                        start=(d2c == 0),
                        stop=(d2c == DC - 1),
                    )
                nc.scalar.activation(
                    AT[src][:, dc, nb * KB : (nb + 1) * KB],
                    ps[:],
                    AF.Identity,
                    bias=ut[:, dc : dc + 1],
                )
        # own projections double as key blocks for the other direction
        # (zaT/zbT are host-permuted so own blocks sit at positions 0, 1);
        # with pT bufs=4 all four survive
        attn_block(own_pts[("b", 0)], "a", first=True)
        attn_block(own_pts[("b", 1)], "a", first=False)
        attn_block(own_pts[("a", 0)], "b", first=True)
        attn_block(own_pts[("a", 1)], "b", first=False)

        # ---------- phase 2: stream the remaining key blocks ----------
        nc.sync.dma_start(wvt[:], _r(wv_d.ap()))
        # key block from src x feeds the OTHER direction's attention
        a_list = list(range(2, NKB))
        b_list = list(range(2, NKB))
        for i in range(len(a_list)):
            pT = proj_block(zT["a"], a_list[i] * KB, f"sa{i}")
            attn_block(pT, "b", first=False)
            if i < len(b_list):
                pT = proj_block(zT["b"], b_list[i] * KB, f"sb{i}")
                attn_block(pT, "a", first=False)

        # ---------- phase 3: finalize ----------
        stream.release()
        final = tc.alloc_tile_pool(name="final", bufs=1)
        # software-pipelined: normalization (DVE) for step i+1 issues ahead
        # of the PE transpose+Wv chain for step i
        steps = [(x, col, qg) for x, col in (("a", 0), ("b", 1))
                 for qg in range(NL // 128)]
        ys = {}

        def norm(step):
            x, _, qg = step
            a = acc[x]
            rr = final.tile([128, 1], dt.float32, tag="rr", bufs=4, name="rr")
            nc.vector.reciprocal(rr[:], a[:, qg, D : D + 1])
            y = final.tile([128, D], F32R, tag="y", bufs=4, name="y")
            nc.vector.tensor_scalar(y[:], a[:, qg, 0:D], rr[:], None, ALU.mult)
            ys[step] = y

        norm(steps[0])
        norm(steps[1])
        for i, (x, col, qg) in enumerate(steps):
            if i + 2 < len(steps):
                norm(steps[i + 2])
            y = ys[(x, col, qg)]
            yt = final.tile([128, DC, 128], F32R, tag="yt", bufs=4, name="yt")
            tp = psum.tile([128, D], F32R, tag="tp", bufs=2, name="tp_f")
            for dc in range(DC):
                nc.tensor.transpose(
                    tp[:, dc * 128 : (dc + 1) * 128],
                    y[:, dc * 128 : (dc + 1) * 128],
                    ident[:],
                )
            nc.scalar.activation(
                yt[:].rearrange("p dc f -> p (dc f)"), tp[:], AF.Copy
            )
            ps = psum.tile([128, D], dt.float32, tag="mm", bufs=4, name="ps_o")
            for dc in range(DC):
                nc.tensor.matmul(
                    ps[:], yt[:, dc, :], wvt[:, dc, :],
                    start=(dc == 0), stop=(dc == DC - 1),
                )
            ot = final.tile([128, D], dt.float32, tag="ot", bufs=3, name="ot")
            nc.vector.tensor_tensor(ot[:], ps[:], bv[:], ALU.add)
            nc.sync.dma_start(
                out_d.ap()[qg * 128 : (qg + 1) * 128, col * D : (col + 1) * D],
                ot[:],
            )

        final.release()
        persist.release()
        const.release()
        psum.release()

    nc.compile()
    return nc


_NC = None


def _get_nc():
    global _NC
    if _NC is None:
        _NC = build()
    return _NC


def _chunk_w(w):
    """[X, Y] -> [128, X//128, Y] partition-chunked, contiguous."""
    x, y = w.shape
    return np.ascontiguousarray(w.reshape(x // 128, 128, y).transpose(1, 0, 2))


def _chunk_b(b):
    return np.ascontiguousarray(np.asarray(b, np.float32).reshape(-1, 128).T)


def prep_in_maps(za, zb, W1, b1, W2, b2, Wq, bq, Wk, bk, Wv, bv):
    za = np.asarray(za, np.float32)
    zb = np.asarray(zb, np.float32)
    W1 = np.asarray(W1, np.float32)
    W2 = np.asarray(W2, np.float32)
    Wq = np.asarray(Wq, np.float32)
    Wk = np.asarray(Wk, np.float32)
    Wv = np.asarray(Wv, np.float32)
    b1 = np.asarray(b1, np.float32)
    b2 = np.asarray(b2, np.float32)
    bq = np.asarray(bq, np.float32)
    bv = np.asarray(bv, np.float32)

    import ml_dtypes
    zdt = ml_dtypes.bfloat16 if W1_BF16 else np.float32

    M = (Wq @ Wk.T) / SCALE
    u = (np.asarray(bq, np.float64) @ np.asarray(Wk, np.float64).T / SCALE).astype(
        np.float32
    )

    zaT = _chunk_w(np.ascontiguousarray(za.T))  # [128, HC, N]
    zbT = _chunk_w(np.ascontiguousarray(zb.T))
    shared = {
        "W1t": _chunk_w(W1).astype(zdt),
        "W2t": _chunk_w(W2),
        "Mt": _chunk_w(M).astype(ml_dtypes.bfloat16),
        "Wvt": _chunk_w(Wv),
        "b1t": _chunk_b(b1),
        "b1p1t": _chunk_b(b1 + 1.0),
        "b2t": _chunk_b(b2 - W2.sum(axis=0)),
        "ut": _chunk_b(u),
        "bvt": np.ascontiguousarray(np.broadcast_to(bv, (128, D)).astype(np.float32)),
        "ident": np.eye(128, dtype=np.float32),
        "identb": np.eye(128, dtype=np.float32).astype(ml_dtypes.bfloat16),
        "vpad": np.ascontiguousarray(
            np.broadcast_to(
                np.tile(np.array([1.0, 0.0], np.float32), DC), (128, 2 * DC)
            )
        ).astype(ml_dtypes.bfloat16),
    }
    # per core: own two 512-row blocks rolled to the front of both zaT and
    # zbT (the kernel reuses its phase-1 own projections as key blocks)
    zaT_blk = zaT.reshape(128, HC, NKB, KB)
    zbT_blk = zbT.reshape(128, HC, NKB, KB)
    in_maps = []
    for c in range(R):
        cs = slice(c * NL, (c + 1) * NL)
        perm = [2 * c, 2 * c + 1] + [i for i in range(NKB) if i // 2 != c]
        in_maps.append(
            {
                "zqa": np.ascontiguousarray(zaT[:, :, cs]).astype(zdt),
                "zqb": np.ascontiguousarray(zbT[:, :, cs]).astype(zdt),
                "zaT": zaT_blk[:, :, perm, :].reshape(128, HC, N).astype(zdt),
                "zbT": zbT_blk[:, :, perm, :].reshape(128, HC, N).astype(zdt),
                **shared,
            }
        )
    return in_maps


def kernel(**inputs) -> np.ndarray:
    nc = _get_nc()
    in_maps = prep_in_maps(**inputs)
    res = run_bass_kernel_spmd(nc, in_maps, core_ids=list(range(R)))
    return np.concatenate([res.results[c]["out"] for c in range(R)], axis=0)
